# revision 39
# baseline (speedup 1.0000x reference)
"""Trainium2 Bass kernel for nn_AlignmentModule (conv stems + L2 score +
log-softmax + beta-binomial prior).

Sharding: 8 cores = 4 batches x 2 T_feats halves. Each core computes the
text conv stem for its batch, its half of the feats conv stem, and the
(T_text, 400) score block in a TRANSPOSED layout (text on partitions,
feats on the free dim), then adds the host-precomputed beta-binomial
prior and writes the transposed block; the host transposes back.

Design notes (vs the fp32r baseline, ~41.6us -> ~31us):
 - all conv/score matmuls in bf16 (1 cycle/row streaming, fast weight
   loads, high PE-array duty that trips the HAM clock to 2.4 GHz; a
   ~3.4us warm-up matmul burst bridges the free-running HAM window
   while the first input DMAs land).
 - |f|^2 enters the d2 PSUM group as ones^T @ (f.f) matmuls (pure f32
   accumulation); |t|^2 (+x_mask penalty) rides the per-partition bias
   port of the epilogue's Ln pass.
 - sqrt(d2) = exp(0.5 ln d2): every ACT function used (exp/ln/relu/
   identity/square) lives in the single natural_log_exp_and_others
   table set -> exactly one ACT table load, warmed during the DMAs.
   (Sqrt would force a second table set and mid-kernel reloads.)
 - log-softmax without max subtraction (scores are in [-20, 0]; exp is
   exact-safe in f32): Z = ones^T exp(-s) via PE, one Ln, one rank-1.
 - the output is assembled entirely in a second PSUM bank: po =
   (-prior, fp16 identity matmul) + (s, fp16 identity matmul) +
   (lnZ, rank-1), so po = -(o); the two chunks evacuate in parallel
   on ACT and DVE as plain copies and the host negates.
 - input packs sized/ordered for the DMA descriptor-round latency
   (~0.5us per 16 partition rows; columns nearly free); conv biases
   are memset to zero on chip (setup_inputs always zeroes them) with a
   bias-DMA program variant compiled on demand if they are nonzero.

Self-contained: hardcodes all shapes; reads nothing from disk.
"""

import math
import os
import subprocess
import sys

import numpy as np
import ml_dtypes

import concourse.bass as bass
import concourse.mybir as mybir
import concourse.tile as tile
from concourse.bass_utils import run_bass_kernel_spmd

B, T_TEXT, T_FEATS = 4, 160, 800
ADIM, ODIM = 256, 80
N_CORES = 8
HALF = T_FEATS // 2          # 400 feats rows per core
TT = T_TEXT                  # 160
TC = TT // 2                 # 80: text chunk (psum partition dim)
TFW = HALF + 2               # 402: f1 window  [s-1, s+401)
TFIN = HALF + 4              # 404: feats input window [s-2, s+402)
TTP = TT + 2                 # 162: text input window (zero halo cols)
F32 = mybir.dt.float32
F32R = mybir.dt.float32r
BF16 = mybir.dt.bfloat16
FP16 = mybir.dt.float16
MASK_PENALTY = 6.0e4         # exp(-sqrt(6e4)) == 0 in f32; fp16-safe

# ---- input pack layouts -------------------------------------------------
# pk_fw (80, PKF_W) bf16: featsT window | fw1
PKF_FEATS = 0
PKF_FW1 = TFIN
PKF_W = TFIN + 3 * 256

# pk_a1 (128, PKA1_W) bf16: textT (2ci x TTP) | tw1 (2ci x 768) | f1 halo masks
PKA1_TEXT = 0
PKA1_TW1 = 2 * TTP
PKA1_MASK = PKA1_TW1 + 2 * 768
PKA1_W = PKA1_MASK + 2

# pk_a2 (128, 1536) bf16: fw2
PKA2_W = 2 * 768

# pk_a3 (128, 1024) bf16: tw2 (2ci x 256) | fw3 (2ci x 256)
PKA3_TW2 = 0
PKA3_FW3 = 2 * 256
PKA3_W = 4 * 256

# pk_p (80, PKP_W) fp16: prior_T (2 chunks x HALF) | I80 | maskpen cols
PKP_PRIOR = 0
PKP_I = 2 * HALF
PKP_MASK = PKP_I + TC
PKP_W = PKP_MASK + 2

# pk_b (128, 12) f32 bias columns
BI_FB1 = 0    # cols 0,1   f_b1 per co chunk
BI_FB2 = 2    # cols 2,3
BI_FB3 = 4    # cols 4,5   (identity and square evacs share it)
BI_TB1 = 6    # cols 6,7
BI_TB2 = 8    # cols 8,9   +t_b2
BI_TB2N = 10  # cols 10,11 -2*t_b2
PKB_W = 12

_nc_cache = {}
_prior_cache = None


# ---------------------------------------------------------------- host math
def _prior_f64():
    """f64 fallback replica of reference.beta_binomial_prior."""
    try:
        from scipy.special import gammaln as _gl
    except Exception:
        _gl = np.vectorize(math.lgamma)
    T, N = float(T_FEATS), float(T_TEXT)
    a = np.arange(1, T_FEATS + 1, dtype=np.float64)
    b = T - a + 1.0
    k = np.arange(T_TEXT, dtype=np.float64)[:, None]

    def betaln(x, y):
        return _gl(x) + _gl(y) - _gl(x + y)

    logp = (
        _gl(N + 1.0) - _gl(k + 1.0) - _gl(N - k + 1.0)
        + betaln(k + a, N - k + b) - betaln(a, b)
    )
    return np.asarray(logp.T, dtype=np.float32)


_NIX_SITE = ("/nix/store/z022hj2nvbm3nwdizlisq4ylc0y7rd6q-python3-3.13.14-env"
             "/lib/python3.13/site-packages")

_PRIOR_SRC = """
import os
os.environ["JAX_PLATFORMS"] = "cpu"
import numpy as np
import jax.numpy as jnp
from jax.scipy.special import gammaln

T, N = {T}, {N}
a = 1.0 * jnp.arange(1, T + 1, dtype=jnp.float32)
b = 1.0 * (T - a + 1.0)
k = jnp.arange(N, dtype=jnp.float32)[:, None]
Nf = jnp.float32(N)

def betaln(x, y):
    return gammaln(x) + gammaln(y) - gammaln(x + y)

logp = (gammaln(Nf + 1.0) - gammaln(k + 1.0) - gammaln(Nf - k + 1.0)
        + betaln(k + a, Nf - k + b) - betaln(a, b))
np.save({out!r}, np.asarray(logp.T, dtype=np.float32))
"""


def _beta_binomial_prior():
    """beta_binomial_prior(T_FEATS, T_TEXT) matching the reference's jax
    f32 computation, via a jax-CPU subprocess (python -S skips the
    sitecustomize that would force the axon/neuron backend). Falls back
    to a scipy f64 replica (abs diff ~1e-3, harmless at the 2e-2 gate)."""
    global _prior_cache
    if _prior_cache is not None:
        return _prior_cache
    cache = f"/tmp/_bbprior_{T_FEATS}x{T_TEXT}.npy"
    if not os.path.exists(cache):
        src = _PRIOR_SRC.format(T=T_FEATS, N=T_TEXT, out=cache)
        for _attempt in range(2):
            try:
                env = dict(os.environ)
                env["JAX_PLATFORMS"] = "cpu"
                env["TRN_TERMINAL_POOL_IPS"] = ""
                env["PYTHONPATH"] = _NIX_SITE
                r = subprocess.run([sys.executable, "-S", "-c", src],
                                   capture_output=True, timeout=120, env=env)
                if r.returncode == 0 and os.path.exists(cache):
                    break
            except Exception:
                pass
    if os.path.exists(cache):
        _prior_cache = np.load(cache).astype(np.float32)
    else:
        _prior_cache = _prior_f64()
    return _prior_cache


# ------------------------------------------------------------- BIR patching
def _split_multiwait(nc):
    """This container's walrus accepts at most one sync wait per
    instruction; move extras onto single-wait NOPs just before."""
    for f in nc.m.functions:
        for bb in f.blocks:
            changed = False
            out = []
            for inst in bb.instructions:
                si = inst.sync_info
                if si is not None and len(si.on_wait) > 1:
                    waits = list(si.on_wait)
                    for j, w in enumerate(waits[:-1]):
                        nop = mybir.InstNoOp(name=f"{inst.name}sw{j}")
                        nop.name = f"{inst.name}sw{j}"
                        nop.engine = inst.engine
                        nop.sync_info = mybir.SyncInfo(on_wait=[w], on_update=[])
                        out.append(nop)
                    inst.sync_info = mybir.SyncInfo(
                        on_wait=[waits[-1]], on_update=list(si.on_update)
                    )
                    changed = True
                out.append(inst)
            if changed:
                bb.instructions = out


# ------------------------------------------------------------ device program
def _build_program(with_biases=False):
    if with_biases in _nc_cache:
        return _nc_cache[with_biases]

    nc = bass.Bass("TRN2", target_bir_lowering=False, debug=False,
                   num_devices=N_CORES, enable_asserts=False)
    AF = mybir.ActivationFunctionType
    AL = mybir.AluOpType

    if with_biases:
        d_pb = nc.dram_tensor("pk_b", [128, PKB_W], F32, kind="ExternalInput")
    d_pf = nc.dram_tensor("pk_fw", [ODIM, PKF_W], BF16, kind="ExternalInput")
    d_a1 = nc.dram_tensor("pk_a1", [128, PKA1_W], BF16, kind="ExternalInput")
    d_a2 = nc.dram_tensor("pk_a2", [128, PKA2_W], BF16, kind="ExternalInput")
    d_a3 = nc.dram_tensor("pk_a3", [128, PKA3_W], BF16, kind="ExternalInput")
    d_pp = nc.dram_tensor("pk_p", [TC, PKP_W], FP16, kind="ExternalInput")
    d_out = nc.dram_tensor("out", [TC, 2 * HALF], F32, kind="ExternalOutput")

    with tile.TileContext(nc) as tc:
        with (
            tc.tile_pool(name="dpool", bufs=1) as dpool,
            tc.tile_pool(name="spool", bufs=1) as spool,
            tc.tile_pool(name="pconv", bufs=3, space="PSUM") as pconv,
            tc.tile_pool(name="psmall", bufs=1, space="PSUM") as psmall,
            tc.tile_pool(name="pd2", bufs=2, space="PSUM") as pd2p,
            tc.tile_pool(name="ppo", bufs=2, space="PSUM") as ppop,
        ):
            # ---------------- input DMAs (need-ordered) -----------------
            # 16-row lead DMA primes the SDMA pipeline so the main
            # transfer's packets overlap HBM latency against a queued
            # descriptor backlog instead of arriving just-in-time
            pf = dpool.tile([ODIM, PKF_W], BF16, name="pf")
            nc.sync.dma_start(pf[0:16, :], d_pf.ap()[0:16, :])
            nc.sync.dma_start(pf[16:ODIM, :], d_pf.ap()[16:ODIM, :])
            a1 = dpool.tile([128, PKA1_W], BF16, name="a1")
            nc.sync.dma_start(a1[:], d_a1.ap())
            a2 = dpool.tile([128, PKA2_W], BF16, name="a2")
            nc.sync.dma_start(a2[:], d_a2.ap())
            a3 = dpool.tile([128, PKA3_W], BF16, name="a3")
            nc.sync.dma_start(a3[:], d_a3.ap())
            pp = dpool.tile([TC, PKP_W], FP16, name="pp")
            nc.scalar.dma_start(pp[:], d_pp.ap())
            pb = dpool.tile([128, PKB_W], F32, name="pb")
            if with_biases:
                nc.scalar.dma_start(pb[:], d_pb.ap())
            else:
                nc.vector.memset(pb[:], 0.0)

            def bias(base, j):
                return pb[:, base + j: base + j + 1]

            # ---------------- on-chip constants + warms -----------------
            ones_blk = spool.tile([128, TC], BF16, name="ones_blk")
            nc.vector.memset(ones_blk[:], 1.0)
            ones_col = ones_blk[:, 0:1]
            pones_row = spool.tile([1, TC], FP16, name="pones_row")
            nc.vector.memset(pones_row[:], 1.0)

            # warm the ACT table slots during the input DMAs; the input
            # is uninitialized scratch so no data deps. sqrt_and_others
            # covers sqrt/square/identity/relu; natural_log_exp covers
            # exp/ln.
            scr = spool.tile([1, 4], F32, name="scr")
            nc.scalar.activation(scr[0:1, 1:2], scr[0:1, 3:4], AF.Exp)
            nc.scalar.activation(scr[0:1, 2:3], scr[0:1, 3:4], AF.Ln)

            # PE HAM warm-up: N=128 keeps the PE array at ~100% duty so
            # the free-running 3.4us HAM window sees a busy period and
            # lifts the clock to 2.4 GHz before the real matmuls.
            wscr = spool.tile([128, 512], BF16, name="wscr")
            nc.vector.memset(wscr[:], 1.0)
            pwarm = pd2p.tile([128, 512], F32, name="pwarm", tag="d2p",
                              padded_shape=[128, 512])

            def warm_mms(n, cols=128):
                for _ in range(n):
                    nc.tensor.matmul(pwarm[:, 0:cols], wscr[:, 0:128],
                                     wscr[:, 0:cols],
                                     start=True, stop=True,
                                     skip_group_check=True)

            warm_mms(9, cols=512)

            # ---------------- feats conv1 -------------------------------
            # psum tiles padded to 402 f32 so the pool rotates uniformly
            f1_sb = []
            for co in range(2):
                p = pconv.tile([128, TFW], F32, name=f"pf1_{co}", tag="convp")
                for k in range(3):
                    nc.tensor.matmul(
                        p[:],
                        pf[:, PKF_FW1 + 256 * k + 128 * co:
                           PKF_FW1 + 256 * k + 128 * (co + 1)],
                        pf[:, k:k + TFW],
                        start=(k == 0), stop=(k == 2),
                    )
                f1 = spool.tile([128, TFW], BF16, name=f"f1_{co}")
                if co == 0:
                    nc.scalar.activation(f1[:], p[:], AF.Relu,
                                         bias=bias(BI_FB1, co))
                else:
                    nc.vector.tensor_scalar(f1[:], p[:], bias(BI_FB1, co), 0.0,
                                            op0=AL.add, op1=AL.max)
                # zero the halo column the reference conv padding zeroes
                nc.vector.tensor_mul(f1[:, 0:1], f1[:, 0:1],
                                     a1[:, PKA1_MASK:PKA1_MASK + 1])
                nc.vector.tensor_mul(f1[:, TFW - 1:TFW], f1[:, TFW - 1:TFW],
                                     a1[:, PKA1_MASK + 1:PKA1_MASK + 2])
                f1_sb.append(f1)

            # ---------------- text conv1 --------------------------------
            t1_sb = []
            for co in range(2):
                p = pconv.tile([128, TT], F32, name=f"pt1_{co}", tag="convp",
                               padded_shape=[128, TFW])
                n = 0
                for ci in range(2):
                    for k in range(3):
                        nc.tensor.matmul(
                            p[:],
                            a1[:, PKA1_TW1 + 768 * ci + 256 * k + 128 * co:
                               PKA1_TW1 + 768 * ci + 256 * k + 128 * (co + 1)],
                            a1[:, TTP * ci + k: TTP * ci + k + TT],
                            start=(n == 0), stop=(n == 5),
                        )
                        n += 1
                t1 = spool.tile([128, TT], BF16, name=f"t1_{co}")
                if co == 0:
                    nc.scalar.activation(t1[:], p[:], AF.Relu,
                                         bias=bias(BI_TB1, co))
                else:
                    nc.vector.tensor_scalar(t1[:], p[:], bias(BI_TB1, co), 0.0,
                                            op0=AL.add, op1=AL.max)
                t1_sb.append(t1)

            # ---------------- text conv2: tneg2 = -2t, tt = t^2 ---------
            tneg2_sb, tt_sb = [], []
            for co in range(2):
                p = pconv.tile([128, TT], F32, name=f"pt2_{co}", tag="convp",
                               padded_shape=[128, TFW])
                for ci in range(2):
                    nc.tensor.matmul(
                        p[:],
                        a3[:, PKA3_TW2 + 256 * ci + 128 * co:
                           PKA3_TW2 + 256 * ci + 128 * (co + 1)],
                        t1_sb[ci][:],
                        start=(ci == 0), stop=(ci == 1),
                    )
                tneg2 = spool.tile([128, TT], BF16, name=f"tneg2_{co}")
                tt = spool.tile([128, TT], BF16, name=f"tt_{co}")
                if co == 0:
                    nc.scalar.activation(tneg2[:], p[:], AF.Identity,
                                         scale=-2.0, bias=bias(BI_TB2N, co))
                else:
                    nc.vector.tensor_scalar(tneg2[:], p[:], bias(BI_TB2, co),
                                            -2.0, op0=AL.add, op1=AL.mult)
                # tt = tneg2^2 (= 4 t^2; the 1/4 is folded into bias_col)
                nc.vector.tensor_mul(tt[:], tneg2[:], tneg2[:])
                tneg2_sb.append(tneg2)
                tt_sb.append(tt)

            # fp16 mask penalty columns -> f32 once (scalar APs must be f32)
            maskf = spool.tile([TC, 2], F32, name="maskf")
            nc.vector.tensor_copy(maskf[:], pp[:, PKP_MASK:PKP_MASK + 2])

            # |t|^2 columns per text chunk: (TC, 1) = tt_chunk^T @ ones
            bias_col = []
            for c in range(2):
                ptn = psmall.tile([TC, 1], F32, name=f"ptn_{c}", tag="smallp",
                                  padded_shape=[TC, HALF])
                for ci in range(2):
                    nc.tensor.matmul(ptn[:],
                                     tt_sb[ci][:, TC * c:TC * (c + 1)],
                                     ones_col[:],
                                     start=(ci == 0), stop=(ci == 1))
                # bc = 0.25*(ptn + pen): tt carries 4*t^2, and the fp16
                # mask penalty column is scaled 0.25x too (still >> enough
                # to zero the exp for masked positions)
                bc = spool.tile([TC, 1], F32, name=f"bias_col_{c}")
                nc.vector.tensor_scalar(bc[:], ptn[:], maskf[:, c:c + 1],
                                        0.25, op0=AL.add, op1=AL.mult)
                bias_col.append(bc)

            # ---------------- feats conv2 -------------------------------
            f2_sb = []
            for co in range(2):
                p = pconv.tile([128, HALF], F32, name=f"pf2_{co}", tag="convp",
                               padded_shape=[128, TFW])
                n = 0
                for ci in range(2):
                    for k in range(3):
                        nc.tensor.matmul(
                            p[:],
                            a2[:, 768 * ci + 256 * k + 128 * co:
                               768 * ci + 256 * k + 128 * (co + 1)],
                            f1_sb[ci][:, k:k + HALF],
                            start=(n == 0), stop=(n == 5),
                        )
                        n += 1
                f2 = spool.tile([128, HALF], BF16, name=f"f2_{co}")
                if co == 0:
                    nc.scalar.activation(f2[:], p[:], AF.Relu,
                                         bias=bias(BI_FB2, co))
                else:
                    nc.vector.tensor_scalar(f2[:], p[:], bias(BI_FB2, co), 0.0,
                                            op0=AL.add, op1=AL.max)
                f2_sb.append(f2)

            # ---------------- feats conv3: f, ff = f^2 ------------------
            f_sb, ff_sb = [], []
            for co in range(2):
                p = pconv.tile([128, HALF], F32, name=f"pf3_{co}", tag="convp",
                               padded_shape=[128, TFW])
                for ci in range(2):
                    nc.tensor.matmul(
                        p[:],
                        a3[:, PKA3_FW3 + 256 * ci + 128 * co:
                           PKA3_FW3 + 256 * ci + 128 * (co + 1)],
                        f2_sb[ci][:],
                        start=(ci == 0), stop=(ci == 1),
                    )
                f = spool.tile([128, HALF], BF16, name=f"f_{co}")
                ff = spool.tile([128, HALF], BF16, name=f"ff_{co}")
                nc.vector.tensor_scalar_add(f[:], p[:], bias(BI_FB3, co))
                nc.scalar.activation(ff[:], p[:], AF.Square,
                                     bias=bias(BI_FB3, co))
                f_sb.append(f)
                ff_sb.append(ff)

            # ---------------- d2 (transposed): (TC, HALF) per chunk -----
            # |f|^2 rides in as ones^T @ ff matmuls replicated across the
            # text partitions -- pure PSUM f32 accumulation, no rank-1s.
            d2ps = []
            for c in range(2):
                p = pd2p.tile([TC, HALF], F32, name=f"pd2_{c}", tag="d2p",
                              padded_shape=[128, 512])
                for ci in range(2):
                    nc.tensor.matmul(p[:],
                                     tneg2_sb[ci][:, TC * c:TC * (c + 1)],
                                     f_sb[ci][:],
                                     start=(ci == 0), stop=False)
                for ci in range(2):
                    nc.tensor.matmul(p[:], ones_blk[:], ff_sb[ci][:],
                                     start=False, stop=(ci == 1))
                d2ps.append(p)

            # prior into po psum (fp16 identity matmul), held to the end
            pos = []
            for c in range(2):
                po = ppop.tile([TC, HALF], F32, name=f"po_{c}", tag="pop")
                nc.tensor.matmul(po[:],
                                 pp[:, PKP_I:PKP_I + TC],
                                 pp[:, PKP_PRIOR + HALF * c:
                                    PKP_PRIOR + HALF * (c + 1)],
                                 start=True, stop=False)
                pos.append(po)

            # ---------------- epilogue ----------------------------------
            pz = psmall.tile([1, HALF], F32, name="pz", tag="smallp",
                             padded_shape=[TC, HALF])
            s_sb = []
            for c in range(2):
                u = spool.tile([TC, HALF], F32, name=f"u_{c}")
                nc.scalar.activation(u[:], d2ps[c][:], AF.Ln,
                                     bias=bias_col[c][:])
                s = spool.tile([TC, HALF], FP16, name=f"s_{c}")
                nc.scalar.activation(s[:], u[:], AF.Exp, scale=0.5)
                e = spool.tile([TC, HALF], BF16, name=f"e_{c}")
                nc.scalar.activation(e[:], s[:], AF.Exp, scale=-1.0)
                nc.tensor.matmul(pz[:], ones_col[0:TC, :], e[:],
                                 start=(c == 0), stop=(c == 1))
                # accumulate +s into po via the fp16 identity (prior is
                # shipped negated; po = -prior + s + lnZ = -o)
                nc.tensor.matmul(pos[c][:], pp[:, PKP_I:PKP_I + TC], s[:],
                                 start=False, stop=False)
                s_sb.append(s)

            lnz = spool.tile([1, HALF], FP16, name="lnz")
            nc.scalar.activation(lnz[:], pz[:], AF.Ln)

            o = spool.tile([TC, 2 * HALF], F32, name="o")
            for c in range(2):
                nc.tensor.matmul(pos[c][:], pones_row[:], lnz[:],
                                 start=False, stop=True)
                # po already holds -(o); plain copies, split across ACT
                # and DVE so the two chunks evacuate in parallel
                if c == 0:
                    nc.scalar.activation(o[:, 0:HALF], pos[c][:], AF.Identity)
                else:
                    nc.vector.tensor_copy(o[:, HALF:2 * HALF], pos[c][:])
                eng = nc.sync if c == 0 else nc.scalar
                eng.dma_start(d_out.ap()[:, HALF * c:HALF * (c + 1)],
                              o[:, HALF * c:HALF * (c + 1)])

    _split_multiwait(nc)
    _nc_cache[with_biases] = nc
    return nc


# ------------------------------------------------------------------ host glue
def _bf16(a):
    return np.asarray(a, np.float32).astype(ml_dtypes.bfloat16)


def _h2(a):
    """(256, X) -> (128, 2X): ci chunk c at columns [c*X, (c+1)*X)."""
    return np.concatenate([a[:128], a[128:]], axis=1)


def _prep_shared(t_w1, t_b1, t_w2, t_b2, f_w1, f_b1, f_w2, f_b2, f_w3, f_b3):
    tw1h = np.asarray(t_w1, np.float32).transpose(1, 2, 0).reshape(ADIM, 3 * ADIM)
    tw2h = np.asarray(t_w2, np.float32)[:, :, 0].T
    fw1h = np.asarray(f_w1, np.float32).transpose(1, 2, 0).reshape(ODIM, 3 * ADIM)
    fw2h = np.asarray(f_w2, np.float32).transpose(1, 2, 0).reshape(ADIM, 3 * ADIM)
    fw3h = np.asarray(f_w3, np.float32)[:, :, 0].T

    pk_a2 = np.ascontiguousarray(_bf16(_h2(fw2h)))
    pk_a3 = np.empty((128, PKA3_W), ml_dtypes.bfloat16)
    pk_a3[:, PKA3_TW2:PKA3_TW2 + 2 * 256] = _bf16(_h2(tw2h))
    pk_a3[:, PKA3_FW3:PKA3_FW3 + 2 * 256] = _bf16(_h2(fw3h))

    pk_b = np.zeros((256, PKB_W // 2), np.float32)
    for j, v in enumerate([f_b1, f_b2, f_b3, t_b1, t_b2,
                           -2.0 * np.asarray(t_b2)]):
        pk_b[:, j] = np.asarray(v, np.float32)

    return {
        "pk_a2": pk_a2,
        "pk_a3": pk_a3,
        "pk_b": np.ascontiguousarray(_h2(pk_b)),   # (128, 12)
        "tw1h2": _bf16(_h2(tw1h)),
        "fw1h": _bf16(fw1h),
    }


def _prep_core_inputs(c, text, feats, x_masks, shared):
    b, h = divmod(c, 2)
    s = h * HALF

    pk_f = np.zeros((ODIM, PKF_W), ml_dtypes.bfloat16)
    lo, hi = max(0, s - 2), min(T_FEATS, s + TFW)
    pk_f[:, lo - (s - 2):hi - (s - 2)] = _bf16(feats[b, lo:hi].T)
    pk_f[:, PKF_FW1:] = shared["fw1h"]

    textT = np.zeros((ADIM, TTP), np.float32)
    textT[:, 1:1 + TT] = text[b].T
    pk_a1 = np.empty((128, PKA1_W), ml_dtypes.bfloat16)
    pk_a1[:, PKA1_TEXT:PKA1_TEXT + 2 * TTP] = _bf16(_h2(textT))
    pk_a1[:, PKA1_TW1:PKA1_TW1 + 2 * 768] = shared["tw1h2"]
    pk_a1[:, PKA1_MASK] = 0.0 if s - 1 < 0 else 1.0
    pk_a1[:, PKA1_MASK + 1] = 0.0 if s + HALF >= T_FEATS else 1.0

    prior = _beta_binomial_prior()[s:s + HALF]               # (400, 160)
    pk_p = np.zeros((TC, PKP_W), np.float16)
    priorT = -prior.T                                        # negated (160,400)
    pk_p[:, 0:HALF] = priorT[0:TC].astype(np.float16)
    pk_p[:, HALF:2 * HALF] = priorT[TC:TT].astype(np.float16)
    pk_p[:, PKP_I:PKP_I + TC] = np.eye(TC, dtype=np.float16)
    # x_mask penalty columns appended per text chunk at assembly below
    return {
        "pk_b": shared["pk_b"],
        "pk_fw": pk_f,
        "pk_a1": pk_a1,
        "pk_a2": shared["pk_a2"],
        "pk_a3": shared["pk_a3"],
        "pk_p": pk_p,
        "_mask": x_masks[b],
    }


def kernel(text, feats, text_lengths, feats_lengths, x_masks,
           t_w1, t_b1, t_w2, t_b2, f_w1, f_b1, f_w2, f_b2, f_w3, f_b3):
    text = np.asarray(text, np.float32)
    feats = np.asarray(feats, np.float32)
    x_masks = np.asarray(x_masks)

    shared = _prep_shared(t_w1, t_b1, t_w2, t_b2,
                          f_w1, f_b1, f_w2, f_b2, f_w3, f_b3)
    with_biases = bool(shared["pk_b"].any())
    nc = _build_program(with_biases)
    in_maps = []
    for c in range(N_CORES):
        m = _prep_core_inputs(c, text, feats, x_masks, shared)
        if not with_biases:
            m.pop("pk_b")
        mask = m.pop("_mask").astype(np.float16)             # (160,)
        m["pk_p"][:, PKP_MASK] = MASK_PENALTY * mask[0:TC]
        m["pk_p"][:, PKP_MASK + 1] = MASK_PENALTY * mask[TC:TT]
        in_maps.append(m)
    res = None
    last_exc = None
    for _attempt in range(3):
        try:
            res = run_bass_kernel_spmd(nc, in_maps,
                                       core_ids=list(range(N_CORES)))
            break
        except Exception as e:   # transient NRT exec-unit flake on cold NEFFs
            last_exc = e
    if res is None:
        raise last_exc

    out = np.empty((B, T_FEATS, T_TEXT), np.float32)
    for c in range(N_CORES):
        b, h = divmod(c, 2)
        blk = -res.results[c]["out"]                         # (80, 800)
        ot = np.concatenate([blk[:, 0:HALF], blk[:, HALF:2 * HALF]], axis=0)
        out[b, h * HALF:(h + 1) * HALF, :] = ot.T            # (400, 160)
    return out
